# revision 5
# baseline (speedup 1.0000x reference)
"""DLTC kernel for Trainium2, 8-core data-parallel.

Shards the batch (16384) across 8 NeuronCores (2048 rows each), runs a
bf16 Bass/Tile kernel per core, gathers full-shape fp32 outputs.

Math (per row b):
  wi    = [||O_t||, ||O_t-O_prev||, <O_t,O_prev>/(||O_t||*||O_prev||+eps)]
  W_t   = sigmoid(2*(w_mlp_W @ wi + w_mlp_b))          # == (tanh(.)+1)/2
  x     = [O_prev; O_t*W_t]
  gates = lstm_Wih @ x + bih + bhh  (i,f,g,o; f unused since c0=0)
  O_t'  = sigmoid(o)*tanh(sigmoid(i)*tanh(g))
  Q'    = (k_W.T @ q_W / sqrt(D)) @ O_t' + k_W.T @ q_b / sqrt(D)
  s_m   = <Q', mem_m>           (+const(b) dropped: softmax-invariant)
  attn  = softmax_m(s)
  mem_a = sum_m attn_m * mem_m
  O_up  = v_W @ mem_a + v_b
  z/r   = sigmoid(Wg1 @ mem_m + Wg2 @ O_up + g_b)   (m = 1..9 only)
  m~    = tanh(Wh1 @ (r*mem_m) + Wh2 @ O_up + h_b)
  new_m = m~ + z*(mem_m - m~)
  out mem = [new_1..new_9, O_up];  outputs (O_up, mem_out, O_up, W_t)
"""

import sys

sys.path.insert(0, "/opt/trn_rl_repo")

from contextlib import ExitStack

import ml_dtypes
import numpy as np

import concourse.bacc as bacc
import concourse.bass as bass
import concourse.mybir as mybir
import concourse.tile as tile
from concourse import bass_utils

BF16 = mybir.dt.bfloat16
F32 = mybir.dt.float32
NP_BF16 = ml_dtypes.bfloat16
AF = mybir.ActivationFunctionType
OP = mybir.AluOpType
AX = mybir.AxisListType

N_CORES = 8
B_FULL, D, ML = 16384, 256, 10
B_LOC = B_FULL // N_CORES          # 2048
R = 128                            # rows per tile
T = B_LOC // R                     # 16 tiles
EPS = 1e-6

_CACHE: dict = {}


def _build_nc():
    nc = bacc.Bacc("TRN2", target_bir_lowering=False, debug=False)

    # ---- DRAM I/O --------------------------------------------------------
    d_ot = nc.dram_tensor("ot", [B_LOC, D], BF16, kind="ExternalInput").ap()
    d_op = nc.dram_tensor("op", [B_LOC, D], BF16, kind="ExternalInput").ap()
    d_mem = nc.dram_tensor("mem", [B_LOC, ML, D], BF16, kind="ExternalInput").ap()

    # weights: lhsT layouts [128, K/128, M]
    d_wihT = nc.dram_tensor("wihT", [128, 4, 1024], BF16, kind="ExternalInput").ap()
    w_names = ["w1T", "wz1T", "wr1T", "wh1T", "wz2T", "wr2T", "wh2T", "wvT"]
    d_w = {n: nc.dram_tensor(n, [128, 2, 256], BF16, kind="ExternalInput").ap()
           for n in w_names}
    d_wmlpT = nc.dram_tensor("wmlpT", [3, 256], BF16, kind="ExternalInput").ap()
    d_ones = nc.dram_tensor("ones", [128, 128], BF16, kind="ExternalInput").ap()
    d_idb = nc.dram_tensor("idb", [128, 128], BF16, kind="ExternalInput").ap()
    d_idf = nc.dram_tensor("idf", [128, 128], F32, kind="ExternalInput").ap()
    # biases fp32: [128, ncol] column layouts
    d_bl = nc.dram_tensor("bl", [128, 8], F32, kind="ExternalInput").ap()
    b_names = ["b2", "c1", "vb", "zb", "rb", "hb"]
    d_b = {n: nc.dram_tensor(n, [128, 2], F32, kind="ExternalInput").ap()
           for n in b_names}

    o_oup = nc.dram_tensor("o_oup", [B_LOC, D], BF16, kind="ExternalOutput").ap()
    o_mem = nc.dram_tensor("o_mem", [B_LOC, ML, D], BF16, kind="ExternalOutput").ap()
    o_wt = nc.dram_tensor("o_wt", [B_LOC, D], BF16, kind="ExternalOutput").ap()

    with tile.TileContext(nc) as tc, ExitStack() as ctx:
        wp = ctx.enter_context(tc.tile_pool(name="weights", bufs=1))
        sp = ctx.enter_context(tc.tile_pool(name="work", bufs=2))
        sp1 = ctx.enter_context(tc.tile_pool(name="work1", bufs=1))
        pp_tr = ctx.enter_context(tc.tile_pool(name="ptr", bufs=2, space="PSUM"))
        pp_big = ctx.enter_context(tc.tile_pool(name="pbig", bufs=3, space="PSUM"))
        pp_l = ctx.enter_context(tc.tile_pool(name="pl", bufs=1, space="PSUM"))
        pp_sm = ctx.enter_context(tc.tile_pool(name="psm", bufs=1, space="PSUM"))

        # ---- load weights once ------------------------------------------
        s_wihT = wp.tile([128, 4, 1024], BF16)
        nc.sync.dma_start(s_wihT[:], d_wihT)
        s_w = {}
        for n in w_names:
            s_w[n] = wp.tile([128, 2, 256], BF16, name=f"s_{n}")
            nc.sync.dma_start(s_w[n][:], d_w[n])
        s_wmlpT = wp.tile([3, 256], BF16)
        nc.sync.dma_start(s_wmlpT[:], d_wmlpT)
        s_ones = wp.tile([128, 128], BF16)
        nc.sync.dma_start(s_ones[:], d_ones)
        s_idb = wp.tile([128, 128], BF16)
        nc.sync.dma_start(s_idb[:], d_idb)
        s_idf = wp.tile([128, 128], F32)
        nc.sync.dma_start(s_idf[:], d_idf)
        s_bl = wp.tile([128, 8], F32)
        nc.sync.dma_start(s_bl[:], d_bl)
        s_b = {}
        for n in b_names:
            s_b[n] = wp.tile([128, 2], F32, name=f"s_{n}")
            nc.sync.dma_start(s_b[n][:], d_b[n])

        for t in range(T):
            rs = t * R

            # ============ phase A: per-row pipeline =======================
            ot_nat = sp.tile([128, D], BF16, tag="ot_nat")
            nc.sync.dma_start(ot_nat[:], d_ot[rs:rs + R, :])
            op_nat = sp.tile([128, D], BF16, tag="op_nat")
            nc.sync.dma_start(op_nat[:], d_op[rs:rs + R, :])

            # stats (norms / dot) in natural layout via free-dim accum
            sq_scr = sp.tile([128, D], BF16, tag="sq_scr")
            n2t = sp.tile([128, 1], F32, tag="n2t")
            nc.scalar.activation(sq_scr[:], ot_nat[:], AF.Square, accum_out=n2t[:])
            sq_scr2 = sp.tile([128, D], BF16, tag="sq_scr2")
            n2p = sp.tile([128, 1], F32, tag="n2p")
            nc.scalar.activation(sq_scr2[:], op_nat[:], AF.Square, accum_out=n2p[:])
            ttr_scr = sp.tile([128, D], BF16, tag="ttr_scr")
            nc.vector.tensor_mul(ttr_scr[:], ot_nat[:], op_nat[:])
            p1 = sp.tile([128, 1], F32, tag="p1")
            nc.vector.reduce_sum(p1[:], ttr_scr[:], axis=AX.XYZW)
            p2 = sp.tile([128, 1], F32, tag="p2")  # 2*<O_t,O_prev>
            nc.vector.tensor_scalar(out=p2[:], in0=p1[:], scalar1=2.0,
                                    scalar2=None, op0=OP.mult)

            stats = sp.tile([128, 4], F32, tag="stats")
            # d2 = (n2t + n2p) - 2*dot  = ||O_t - O_prev||^2
            d2 = sp.tile([128, 1], F32, tag="d2")
            nc.vector.scalar_tensor_tensor(
                out=d2[:], in0=n2t[:], scalar=n2p[:], in1=p2[:],
                op0=OP.add, op1=OP.subtract)
            nc.scalar.activation(stats[:, 0:1], n2t[:], AF.Sqrt)        # n_t
            rd_t = sp.tile([128, 1], F32, tag="rd_t")
            nc.scalar.activation(rd_t[:], d2[:], AF.Relu)
            nc.scalar.activation(stats[:, 1:2], rd_t[:], AF.Sqrt)       # rho_d
            np_c = sp.tile([128, 1], F32, tag="np_c")
            nc.scalar.activation(np_c[:], n2p[:], AF.Sqrt)              # n_p
            den = sp.tile([128, 1], F32, tag="den")
            nc.vector.tensor_mul(den[:], stats[:, 0:1], np_c[:])
            den2 = sp.tile([128, 1], F32, tag="den2")
            nc.scalar.activation(den2[:], den[:], AF.Copy, bias=2.0 * EPS, scale=2.0)
            rde = sp.tile([128, 1], F32, tag="rde")
            nc.vector.reciprocal(rde[:], den2[:])
            nc.vector.tensor_mul(stats[:, 2:3], p2[:], rde[:])          # rho_a

            # wiT = stats[:, 0:3].T  -> [3, 128] bf16
            pstf = pp_tr.tile([3, 128], F32, tag="pst", name="pstf")
            nc.tensor.transpose(pstf[:], stats[:, 0:3], s_idf[:])
            wiT = sp.tile([3, 128], BF16, tag="wiT")
            nc.vector.tensor_copy(wiT[:], pstf[:])

            # W_t = sigmoid(w2 @ wi + b2)   (layout B)
            pwt = pp_sm.tile([128, 2, 128], F32, tag="psm", name="pwt")
            for mt in range(2):
                nc.tensor.matmul(pwt[:, mt, :], s_wmlpT[:, mt * 128:(mt + 1) * 128],
                                 wiT[:], start=True, stop=True)
            wtB = sp.tile([128, 2, 128], BF16, tag="wtB")
            for mt in range(2):
                nc.scalar.activation(wtB[:, mt, :], pwt[:, mt, :], AF.Sigmoid,
                                     bias=s_b["b2"][:, mt:mt + 1])

            # transposes of O_t, O_prev -> layout B
            otT = sp.tile([128, 2, 128], BF16, tag="otT")
            opT = sp.tile([128, 2, 128], BF16, tag="opT")
            for src, dst in ((ot_nat, otT), (op_nat, opT)):
                pst = pp_tr.tile([128, 2, 128], BF16, tag="pst", name="pst")
                for kd in range(2):
                    nc.tensor.transpose(pst[:, kd, :],
                                        src[:, kd * 128:(kd + 1) * 128], s_idb[:])
                nc.vector.tensor_copy(dst[:], pst[:])

            otwB = sp.tile([128, 2, 128], BF16, tag="otwB")
            nc.vector.tensor_mul(otwB[:], otT[:], wtB[:])

            # LSTM gates (skip f): mtiles {0,1,4,5,6,7}
            pl0 = pp_l.tile([128, 6, 128], F32, tag="pl")  # i0 i1 g0 g1 o0 o1
            rhs_kt = [opT[:, 0, :], opT[:, 1, :], otwB[:, 0, :], otwB[:, 1, :]]
            mts = [(0, 0), (1, 1), (4, 2), (5, 3), (6, 4), (7, 5)]
            for mt, j in mts:
                for kt in range(4):
                    nc.tensor.matmul(pl0[:, j, :],
                                     s_wihT[:, kt, mt * 128:(mt + 1) * 128],
                                     rhs_kt[kt], start=(kt == 0), stop=(kt == 3))
            si = sp.tile([128, 2, 128], BF16, tag="si")
            tg = sp.tile([128, 2, 128], BF16, tag="tg")
            so = sp.tile([128, 2, 128], BF16, tag="so")
            for j in range(2):
                nc.scalar.activation(si[:, j, :], pl0[:, j, :], AF.Sigmoid,
                                     bias=s_bl[:, j:j + 1])
                nc.scalar.activation(tg[:, j, :], pl0[:, 2 + j, :], AF.Tanh,
                                     bias=s_bl[:, 4 + j:5 + j])
                nc.scalar.activation(so[:, j, :], pl0[:, 4 + j, :], AF.Sigmoid,
                                     bias=s_bl[:, 6 + j:7 + j])
            cc = sp.tile([128, 2, 128], BF16, tag="cc")
            nc.vector.tensor_mul(cc[:], si[:], tg[:])
            tcc = sp.tile([128, 2, 128], BF16, tag="tcc")
            nc.scalar.activation(tcc[:], cc[:], AF.Tanh)
            opr = sp.tile([128, 2, 128], BF16, tag="opr")   # O_t'
            nc.vector.tensor_mul(opr[:], so[:], tcc[:])

            # Q' = W1 @ O_t' + c1
            pq = pp_sm.tile([128, 2, 128], F32, tag="psm", name="pq")
            for mt in range(2):
                for kt in range(2):
                    nc.tensor.matmul(pq[:, mt, :],
                                     s_w["w1T"][:, kt, mt * 128:(mt + 1) * 128],
                                     opr[:, kt, :], start=(kt == 0), stop=(kt == 1))
            qpB = sp.tile([128, 2, 128], BF16, tag="qpB")
            for mt in range(2):
                nc.vector.tensor_scalar(
                    out=qpB[:, mt, :], in0=pq[:, mt, :],
                    scalar1=s_b["c1"][:, mt:mt + 1], scalar2=None, op0=OP.add)

            # ============ phase B: memory pipeline ========================
            mem_nat = sp.tile([128, ML, D], BF16, tag="mem_nat")
            nc.sync.dma_start(mem_nat[:], d_mem[rs:rs + R, :, :])

            memT = sp.tile([128, 2, ML, 128], BF16, tag="memT")
            for kd in range(2):
                for mg0, mgn in ((0, 4), (4, 4), (8, 2)):
                    pst = pp_tr.tile([128, 4, 128], BF16, tag="pst")
                    for j in range(mgn):
                        nc.tensor.transpose(
                            pst[:, j, :],
                            mem_nat[:, mg0 + j, kd * 128:(kd + 1) * 128], s_idb[:])
                    nc.vector.tensor_copy(memT[:, kd, mg0:mg0 + mgn, :],
                                          pst[:, 0:mgn, :])

            # scores: qk = memT * Q'(bcast over m); reduce d via ones-matmul
            qk = sp.tile([128, 2, ML, 128], BF16, tag="qk")
            for kd in range(2):
                nc.vector.tensor_mul(
                    qk[:, kd], memT[:, kd],
                    qpB[:, kd:kd + 1, :].broadcast_to((128, ML, 128)))
            exps = sp.tile([128, ML, 128], BF16, tag="exps")
            for mg0, mgn in ((0, 4), (4, 4), (8, 2)):
                psc = pp_big.tile([128, 4, 128], F32, tag="pbig")
                for kd in range(2):
                    nc.tensor.matmul(psc[:, 0:mgn, :], s_ones[:],
                                     qk[:, kd, mg0:mg0 + mgn, :],
                                     start=(kd == 0), stop=(kd == 1))
                nc.scalar.activation(exps[:, mg0:mg0 + mgn, :], psc[:, 0:mgn, :],
                                     AF.Exp)

            dsum = sp.tile([128, 128], F32, tag="dsum")
            nc.vector.reduce_sum(dsum[:], exps.transpose([0, 2, 1]), axis=AX.X)
            rden = sp.tile([128, 128], F32, tag="rden")
            nc.vector.reciprocal_approx_fast(rden[:], dsum[:])

            # mem_attn = (sum_m exps*mem) * rden
            mau = sp.tile([128, 2, 128], F32, tag="mau")
            for kd in range(2):
                prodm = sp.tile([128, ML, 128], BF16, tag="prodm")
                nc.vector.tensor_mul(prodm[:], memT[:, kd], exps[:])
                nc.vector.reduce_sum(mau[:, kd, :], prodm.transpose([0, 2, 1]),
                                     axis=AX.X)
            maB = sp.tile([128, 2, 128], BF16, tag="maB")
            for kd in range(2):
                nc.vector.tensor_mul(maB[:, kd, :], mau[:, kd, :], rden[:])

            # O_up = v_W @ mem_attn + v_b
            pou = pp_sm.tile([128, 2, 128], F32, tag="psm", name="pou")
            for mt in range(2):
                for kt in range(2):
                    nc.tensor.matmul(pou[:, mt, :],
                                     s_w["wvT"][:, kt, mt * 128:(mt + 1) * 128],
                                     maB[:, kt, :], start=(kt == 0), stop=(kt == 1))
            oupB = sp.tile([128, 2, 128], BF16, tag="oupB")
            for mt in range(2):
                nc.scalar.activation(oupB[:, mt, :], pou[:, mt, :], AF.Identity,
                                     bias=s_b["vb"][:, mt:mt + 1])

            # natural-layout outputs: O_up, W_t
            oup_nat = sp.tile([128, D], BF16, tag="oup_nat")
            wt_nat = sp.tile([128, D], BF16, tag="wt_nat")
            for srcB, dst in ((oupB, oup_nat), (wtB, wt_nat)):
                pst = pp_tr.tile([128, 2, 128], BF16, tag="pst", name="pst")
                for kd in range(2):
                    nc.tensor.transpose(pst[:, kd, :], srcB[:, kd, :], s_idb[:])
                nc.vector.tensor_copy(
                    dst.rearrange("p (k b) -> p k b", k=2), pst[:])
            nc.sync.dma_start(o_oup[rs:rs + R, :], oup_nat[:])
            nc.sync.dma_start(o_mem[rs:rs + R, ML - 1, :], oup_nat[:])
            nc.sync.dma_start(o_wt[rs:rs + R, :], wt_nat[:])

            # gate second terms: Wg2 @ O_up  (f32 in SBUF)
            g2B = {}
            for gn, wn in (("z", "wz2T"), ("r", "wr2T"), ("h", "wh2T")):
                pg2 = pp_sm.tile([128, 2, 128], F32, tag="psm", name="pg2")
                for mt in range(2):
                    for kt in range(2):
                        nc.tensor.matmul(pg2[:, mt, :],
                                         s_w[wn][:, kt, mt * 128:(mt + 1) * 128],
                                         oupB[:, kt, :],
                                         start=(kt == 0), stop=(kt == 1))
                g2B[gn] = sp.tile([128, 2, 128], F32, tag=f"g2B_{gn}",
                                  name=f"g2B_{gn}")
                nc.vector.tensor_copy(g2B[gn][:], pg2[:])

            # z, r gates over m=1..9
            zB = sp1.tile([128, 2, 9, 128], BF16, tag="zB")
            rB = sp1.tile([128, 2, 9, 128], BF16, tag="rB")
            for gn, wn, bn, gout, af in (("z", "wz1T", "zb", zB, AF.Sigmoid),
                                         ("r", "wr1T", "rb", rB, AF.Sigmoid)):
                gpre = sp1.tile([128, 2, 9, 128], BF16, tag="gpre")
                for mt in range(2):
                    for mg0, mgn in ((1, 4), (5, 4), (9, 1)):
                        pg = pp_big.tile([128, 4, 128], F32, tag="pbig")
                        for kt in range(2):
                            nc.tensor.matmul(
                                pg[:, 0:mgn, :],
                                s_w[wn][:, kt, mt * 128:(mt + 1) * 128],
                                memT[:, kt, mg0:mg0 + mgn, :],
                                start=(kt == 0), stop=(kt == 1))
                        nc.vector.tensor_add(
                            gpre[:, mt, mg0 - 1:mg0 - 1 + mgn, :], pg[:, 0:mgn, :],
                            g2B[gn][:, mt:mt + 1, :].broadcast_to((128, mgn, 128)))
                for mt in range(2):
                    nc.scalar.activation(gout[:, mt], gpre[:, mt], af,
                                         bias=s_b[bn][:, mt:mt + 1])

            # h gate: rhs = r*mem
            rmem = sp1.tile([128, 2, 9, 128], BF16, tag="rmem")
            for kd in range(2):
                nc.vector.tensor_mul(rmem[:, kd], rB[:, kd], memT[:, kd, 1:ML, :])
            htB = sp1.tile([128, 2, 9, 128], BF16, tag="htB")
            hpre = sp1.tile([128, 2, 9, 128], BF16, tag="gpre")
            for mt in range(2):
                for mg0, mgn in ((1, 4), (5, 4), (9, 1)):
                    pg = pp_big.tile([128, 4, 128], F32, tag="pbig")
                    for kt in range(2):
                        nc.tensor.matmul(
                            pg[:, 0:mgn, :],
                            s_w["wh1T"][:, kt, mt * 128:(mt + 1) * 128],
                            rmem[:, kt, mg0 - 1:mg0 - 1 + mgn, :],
                            start=(kt == 0), stop=(kt == 1))
                    nc.vector.tensor_add(
                        hpre[:, mt, mg0 - 1:mg0 - 1 + mgn, :], pg[:, 0:mgn, :],
                        g2B["h"][:, mt:mt + 1, :].broadcast_to((128, mgn, 128)))
            for mt in range(2):
                nc.scalar.activation(htB[:, mt], hpre[:, mt], AF.Tanh,
                                     bias=s_b["hb"][:, mt:mt + 1])

            # combine: new = m~ + z*(mem - m~)
            newB = sp1.tile([128, 2, 9, 128], BF16, tag="newB")
            for kd in range(2):
                dsub = sp1.tile([128, 9, 128], BF16, tag="dsub")
                nc.vector.tensor_sub(dsub[:], memT[:, kd, 1:ML, :], htB[:, kd])
                esub = sp1.tile([128, 9, 128], BF16, tag="esub")
                nc.vector.tensor_mul(esub[:], zB[:, kd], dsub[:])
                nc.vector.tensor_add(newB[:, kd], htB[:, kd], esub[:])

            # transpose back to natural, store
            mnew = sp.tile([128, 9, D], BF16, tag="mnew")
            for kd in range(2):
                for mg0, mgn in ((0, 4), (4, 4), (8, 1)):
                    pst = pp_tr.tile([128, 4, 128], BF16, tag="pst")
                    for j in range(mgn):
                        nc.tensor.transpose(pst[:, j, :], newB[:, kd, mg0 + j, :],
                                            s_idb[:])
                    nc.vector.tensor_copy(
                        mnew[:, mg0:mg0 + mgn, kd * 128:(kd + 1) * 128],
                        pst[:, 0:mgn, :])
            nc.sync.dma_start(o_mem[rs:rs + R, 0:ML - 1, :], mnew[:])

    nc.compile()
    return nc


def _get_nc():
    if "nc" not in _CACHE:
        _CACHE["nc"] = _build_nc()
    return _CACHE["nc"]


def _host_prep(inputs):
    f32 = np.float32
    O_t = np.asarray(inputs["O_t"], f32)
    O_prev = np.asarray(inputs["O_prev"], f32)
    memory = np.asarray(inputs["memory"], f32)
    w_mlp_W = np.asarray(inputs["w_mlp_W"], f32)
    w_mlp_b = np.asarray(inputs["w_mlp_b"], f32)
    Wih = np.asarray(inputs["lstm_Wih"], f32)
    bih = np.asarray(inputs["lstm_bih"], f32)
    bhh = np.asarray(inputs["lstm_bhh"], f32)
    q_W = np.asarray(inputs["q_W"], f32)
    q_b = np.asarray(inputs["q_b"], f32)
    k_W = np.asarray(inputs["k_W"], f32)
    k_b = np.asarray(inputs["k_b"], f32)  # noqa: F841  (softmax-invariant)
    v_W = np.asarray(inputs["v_W"], f32)
    v_b = np.asarray(inputs["v_b"], f32)
    z_W = np.asarray(inputs["z_W"], f32)
    z_b = np.asarray(inputs["z_b"], f32)
    r_W = np.asarray(inputs["r_W"], f32)
    r_b = np.asarray(inputs["r_b"], f32)
    h_W = np.asarray(inputs["h_W"], f32)
    h_b = np.asarray(inputs["h_b"], f32)

    inv_sqrt_d = 1.0 / np.sqrt(np.float32(D))

    def lhsT_pack(wT):  # [K, M] -> [128, K//128, M]
        K, M = wT.shape
        return np.ascontiguousarray(
            wT.reshape(K // 128, 128, M).transpose(1, 0, 2)).astype(NP_BF16)

    def bias_pack(b):  # [N] -> [128, N//128] fp32
        return np.ascontiguousarray(b.reshape(-1, 128).T).astype(f32)

    W1 = (k_W.T @ q_W) * inv_sqrt_d          # Q' = W1 @ O_t' + c1
    c1 = (k_W.T @ q_b) * inv_sqrt_d

    shared = {
        "wihT": lhsT_pack(Wih.T),            # [512, 1024]
        "w1T": lhsT_pack(W1.T),
        "wz1T": lhsT_pack(z_W[:, :D].T),
        "wr1T": lhsT_pack(r_W[:, :D].T),
        "wh1T": lhsT_pack(h_W[:, :D].T),
        "wz2T": lhsT_pack(z_W[:, D:].T),
        "wr2T": lhsT_pack(r_W[:, D:].T),
        "wh2T": lhsT_pack(h_W[:, D:].T),
        "wvT": lhsT_pack(v_W.T),
        "wmlpT": np.ascontiguousarray((2.0 * w_mlp_W).T).astype(NP_BF16),
        "ones": np.ones((128, 128), NP_BF16),
        "idb": np.eye(128, dtype=NP_BF16),
        "idf": np.eye(128, dtype=f32),
        "bl": bias_pack(bih + bhh),
        "b2": bias_pack(2.0 * w_mlp_b),
        "c1": bias_pack(c1),
        "vb": bias_pack(v_b),
        "zb": bias_pack(z_b),
        "rb": bias_pack(r_b),
        "hb": bias_pack(h_b),
    }

    in_maps = []
    for c in range(N_CORES):
        s = slice(c * B_LOC, (c + 1) * B_LOC)
        m = dict(shared)
        m["ot"] = O_t[s].astype(NP_BF16)
        m["op"] = O_prev[s].astype(NP_BF16)
        m["mem"] = memory[s].astype(NP_BF16)
        in_maps.append(m)
    return in_maps


def kernel(**inputs):
    in_maps = _host_prep(inputs)
    nc = _get_nc()
    res = bass_utils.run_bass_kernel_spmd(nc, in_maps, core_ids=list(range(N_CORES)))
    O_up = np.concatenate(
        [np.asarray(r["o_oup"], np.float32) for r in res.results], axis=0)
    mem_new = np.concatenate(
        [np.asarray(r["o_mem"], np.float32) for r in res.results], axis=0)
    W_t = np.concatenate(
        [np.asarray(r["o_wt"], np.float32) for r in res.results], axis=0)
    return O_up, mem_new, O_up, W_t
